# revision 6
# baseline (speedup 1.0000x reference)
"""Trainium2 Bass kernel for GQA (nn_GQA_28561532518475).

Strategy
--------
8 cores = 4 batches x 2 kv-head halves. Core c handles batch c//2 and kv heads
[3*(c%2), 3*(c%2)+3) together with their 6 grouped q heads.

The head permutation `perm` is folded into the weights on the host:
  - Wq columns are reordered into group-major "slot" order (slot j = head
    perm[j]) and pre-scaled by HD**-0.5,
  - Wk/Wv rows are reordered by argsort(perm) (equivalent to shuffling x),
  - Wp rows are reordered by perm (undoes the output permutation).
so the device kernel is a plain GQA with no gathers.

On device (per core, all fp32):
  xT [768,2048] and the weight shards are DMA'd to SBUF once.  qT and a
  partition-duplicated kT are computed in transposed layout [hd, P]; v in
  natural layout [P, hd] with an appended ones column (the ones column makes
  the attention matmul also produce the softmax denominator).  Attention per
  (kv head, q head, q-block of 1024): scores_T = kT.T @ qT -> PSUM, exp on ACT
  (scores are O(7), no max subtraction needed) -> SBUF, then out = exp_T.T @
  [v|1] accumulated over the 16 kv row tiles in PSUM.  Normalize by the
  reciprocal of the ones-column, PE-transpose back to [hd, P], project with
  Wp.  Partial outputs (one per half) are summed on the host with the bias.
"""

import numpy as np

B, P, C = 4, 2048, 768
H, HK, HD, GS = 12, 6, 64, 2
SCALE = HD ** -0.5
NKV = 3          # kv heads per core
KT = C // 128    # 6 contraction tiles
PT = P // 128    # 16 row tiles
QB = 1024        # q-block width for attention
NQB = P // QB    # 2
TQ = QB // 128   # 8 q-tiles per block

_cached_nc = None


def _build_program():
    global _cached_nc
    if _cached_nc is not None:
        return _cached_nc

    import concourse.bass as bass  # noqa: F401
    import concourse.mybir as mybir
    import concourse.tile as tile
    from concourse import bacc
    from concourse.masks import make_identity

    fp32 = mybir.dt.float32
    EXP = mybir.ActivationFunctionType.Exp

    nc = bacc.Bacc("TRN2", target_bir_lowering=False, debug=False)
    xT = nc.dram_tensor("xT", [C, P], fp32, kind="ExternalInput").ap()
    wq = nc.dram_tensor("wq", [C, 384], fp32, kind="ExternalInput").ap()
    wkd = nc.dram_tensor("wkd", [C, 384], fp32, kind="ExternalInput").ap()
    wv = nc.dram_tensor("wv", [C, 192], fp32, kind="ExternalInput").ap()
    wp = nc.dram_tensor("wp", [384, C], fp32, kind="ExternalInput").ap()
    y = nc.dram_tensor("y", [P, C], fp32, kind="ExternalOutput").ap()

    with tile.TileContext(nc) as tc:
        from contextlib import ExitStack

        with ExitStack() as ctx:
            wpool = ctx.enter_context(tc.tile_pool(name="weights", bufs=1))
            const = ctx.enter_context(tc.tile_pool(name="const", bufs=1))
            qkvp = ctx.enter_context(tc.tile_pool(name="qkv", bufs=1))
            outp = ctx.enter_context(tc.tile_pool(name="outT", bufs=1))
            ysbp = ctx.enter_context(tc.tile_pool(name="ysb", bufs=3))
            nrmp = ctx.enter_context(tc.tile_pool(name="norm", bufs=4))

            ident = const.tile([128, 128], fp32)
            make_identity(nc, ident)
            # zeros for PSUM-bank zeroing matmuls (start=True clears the
            # whole bank's has_written bits, so packed accumulation regions
            # must share a single start)
            zsb = const.tile([128, 260], fp32)
            nc.vector.memset(zsb[:], 0.0)

            wq_sb = wpool.tile([128, KT, 384], fp32)
            nc.sync.dma_start(wq_sb[:], wq.rearrange("(t p) n -> p t n", p=128))
            wkd_sb = wpool.tile([128, KT, 384], fp32)
            nc.sync.dma_start(wkd_sb[:], wkd.rearrange("(t p) n -> p t n", p=128))
            wv_sb = wpool.tile([128, KT, 192], fp32)
            nc.sync.dma_start(wv_sb[:], wv.rearrange("(t p) n -> p t n", p=128))
            wp_sb = wpool.tile([128, 3, C], fp32)
            nc.sync.dma_start(wp_sb[:], wp.rearrange("(t p) n -> p t n", p=128))

            # per-kv-head tiles so attention can start as soon as its head is
            # projected
            qts = [qkvp.tile([128, P], fp32, name=f"qt{m}", tag=f"qt{m}") for m in range(NKV)]
            kts = [qkvp.tile([128, P], fp32, name=f"kt{m}", tag=f"kt{m}") for m in range(NKV)]
            vexts = [qkvp.tile([128, PT, HD + 1], fp32, name=f"vx{m}", tag=f"vx{m}") for m in range(NKV)]
            for m in range(NKV):
                nc.vector.memset(vexts[m][:, :, HD], 1.0)
            outTs = [outp.tile([128, P], fp32, name=f"oT{m}", tag=f"oT{m}") for m in range(NKV)]

            # ---------------- phase 1: QKV projections ----------------
            with tc.tile_pool(name="xt", bufs=1) as xpool, tc.tile_pool(
                name="qkv_ps", bufs=3, space="PSUM"
            ) as qps:
                xt = xpool.tile([128, KT, P], fp32)
                for kc in range(KT):
                    nc.sync.dma_start(
                        xt[:, kc, :], xT[kc * 128 : (kc + 1) * 128, :]
                    )

                for w_sb, dests in ((wq_sb, qts), (wkd_sb, kts)):
                    for m in range(NKV):
                        for nb in range(4):
                            ps = qps.tile([128, 512], fp32, tag="proj")
                            for kc in range(KT):
                                nc.tensor.matmul(
                                    ps[:],
                                    w_sb[:, kc, m * 128 : (m + 1) * 128],
                                    xt[:, kc, nb * 512 : (nb + 1) * 512],
                                    start=(kc == 0),
                                    stop=(kc == KT - 1),
                                )
                            nc.vector.tensor_copy(
                                dests[m][:, nb * 512 : (nb + 1) * 512], ps[:]
                            )

                for i in range(PT):
                    ps = qps.tile([128, 192], fp32, tag="vproj")
                    for kc in range(KT):
                        nc.tensor.matmul(
                            ps[:],
                            xt[:, kc, i * 128 : (i + 1) * 128],
                            wv_sb[:, kc, :],
                            start=(kc == 0),
                            stop=(kc == KT - 1),
                        )
                    psv = ps.rearrange("p (h d) -> p h d", h=NKV)
                    for m in range(NKV):
                        nc.vector.tensor_copy(vexts[m][:, i, 0:HD], psv[:, m, :])

            # ---------------- phase 2: attention ----------------
            with tc.tile_pool(name="E", bufs=3) as epool, tc.tile_pool(
                name="nst", bufs=2
            ) as nstp, tc.tile_pool(
                name="s_ps", bufs=2, space="PSUM"
            ) as sps, tc.tile_pool(
                name="o_ps", bufs=1, space="PSUM"
            ) as ops, tc.tile_pool(
                name="t_ps", bufs=2, space="PSUM"
            ) as tps:
                for kv in range(NKV):
                    for jq in range(NQB):
                        q0 = jq * QB
                        # staging for normalized out, both heads: cols g*64..
                        nsts = [
                            nstp.tile([128, 128], fp32, name=f"nst{t}", tag=f"nst{t}")
                            for t in range(TQ)
                        ]
                        for g in range(GS):
                            gp = slice(g * 64, (g + 1) * 64)
                            o_a = ops.tile([128, 4 * 65], fp32, tag="oa")
                            o_b = ops.tile([128, 4 * 65], fp32, tag="ob")
                            nc.tensor.matmul(
                                o_a[:], ident[:], zsb[:], start=True, stop=False
                            )
                            nc.tensor.matmul(
                                o_b[:], ident[:], zsb[:], start=True, stop=False
                            )
                            for i in range(PT):
                                s = sps.tile([128, QB], fp32, tag="s")
                                for nb in range(QB // 512):
                                    nc.tensor.matmul(
                                        s[:, nb * 512 : (nb + 1) * 512],
                                        kts[kv][gp, i * 128 : (i + 1) * 128],
                                        qts[kv][gp, q0 + nb * 512 : q0 + (nb + 1) * 512],
                                        start=True,
                                        stop=True,
                                    )
                                e = epool.tile([128, QB], fp32, tag="e")
                                nc.scalar.activation(e[:], s[:], EXP)
                                for t in range(TQ):
                                    dst = o_a if t < 4 else o_b
                                    off = (t % 4) * 65
                                    nc.tensor.matmul(
                                        dst[:, off : off + 65],
                                        e[:, t * 128 : (t + 1) * 128],
                                        vexts[kv][:, i, :],
                                        start=False,
                                        stop=(i == PT - 1 and t % 4 == 3),
                                        skip_group_check=True,
                                    )
                            for t in range(TQ):
                                src = o_a if t < 4 else o_b
                                off = (t % 4) * 65
                                r = nrmp.tile([128, 1], fp32, tag="r")
                                nc.vector.reciprocal(r[:], src[:, off + 64 : off + 65])
                                nc.vector.tensor_scalar_mul(
                                    nsts[t][:, g * 64 : (g + 1) * 64],
                                    src[:, off : off + 64],
                                    r[:],
                                )
                        for t in range(TQ):
                            tp = tps.tile([128, 128], fp32, tag="t")
                            nc.tensor.transpose(tp[:], nsts[t][:], ident[:])
                            nc.vector.tensor_copy(
                                outTs[kv][:, q0 + t * 128 : q0 + (t + 1) * 128],
                                tp[:],
                            )

            # ---------------- phase 3: output projection ----------------
            with tc.tile_pool(name="y_ps", bufs=4, space="PSUM") as yps:
                for mt in range(PT):
                    for nh in range(2):
                        ps = yps.tile([128, 384], fp32, tag="y")
                        for kf in range(3):
                            nc.tensor.matmul(
                                ps[:],
                                outTs[kf][:, mt * 128 : (mt + 1) * 128],
                                wp_sb[:, kf, nh * 384 : (nh + 1) * 384],
                                start=(kf == 0),
                                stop=(kf == 2),
                            )
                        ysb = ysbp.tile([128, 384], fp32, tag="y")
                        nc.vector.tensor_copy(ysb[:], ps[:])
                        nc.sync.dma_start(
                            y[mt * 128 : (mt + 1) * 128, nh * 384 : (nh + 1) * 384],
                            ysb[:],
                        )

    nc.compile()
    _cached_nc = nc
    return nc


def _make_in_maps(x, Wq, Wk, Wv, Wp, perm):
    inv = np.argsort(perm)
    Wq_f = np.ascontiguousarray(
        Wq.reshape(C, H, HD)[:, perm, :].reshape(C, C) * SCALE
    )
    Wk_f = np.ascontiguousarray(Wk.reshape(H, HD, HK * HD)[inv].reshape(C, HK * HD))
    Wv_f = np.ascontiguousarray(Wv.reshape(H, HD, HK * HD)[inv].reshape(C, HK * HD))
    Wp_f = np.ascontiguousarray(Wp.reshape(H, HD, C)[perm].reshape(C, C))

    in_maps = []
    for core in range(8):
        b, half = core // 2, core % 2
        wk_half = Wk_f[:, half * 192 : (half + 1) * 192].reshape(C, NKV, 1, HD)
        wkd = np.ascontiguousarray(
            np.broadcast_to(wk_half, (C, NKV, 2, HD)).reshape(C, 384)
        )
        in_maps.append(
            {
                "xT": np.ascontiguousarray(x[b].T),
                "wq": np.ascontiguousarray(Wq_f[:, half * 384 : (half + 1) * 384]),
                "wkd": wkd,
                "wv": np.ascontiguousarray(Wv_f[:, half * 192 : (half + 1) * 192]),
                "wp": np.ascontiguousarray(Wp_f[half * 384 : (half + 1) * 384, :]),
            }
        )
    return in_maps


def kernel(x, Wq, Wk, Wv, Wp, bp, bass_run_kwargs=None, **_unused):
    perm = _unused.pop("perm")
    from concourse.bass_utils import run_bass_kernel_spmd

    x = np.asarray(x, np.float32)
    nc = _build_program()
    in_maps = _make_in_maps(
        x,
        np.asarray(Wq, np.float32),
        np.asarray(Wk, np.float32),
        np.asarray(Wv, np.float32),
        np.asarray(Wp, np.float32),
        np.asarray(perm),
    )
    res = run_bass_kernel_spmd(
        nc, in_maps, core_ids=list(range(8)), **(bass_run_kwargs or {})
    )
    bp = np.asarray(bp, np.float32)
    y = np.empty((B, P, C), np.float32)
    for b in range(B):
        y[b] = res.results[2 * b]["y"] + res.results[2 * b + 1]["y"] + bp
    if bass_run_kwargs:
        kernel.last_results = res
    return y


# revision 10
# speedup vs baseline: 1.7378x; 1.7378x over previous
"""Trainium2 Bass kernel for GQA (nn_GQA_28561532518475).

Strategy
--------
8 cores = 4 batches x 2 kv-head halves. Core c handles batch c//2 and kv heads
[3*(c%2), 3*(c%2)+3) together with their 6 grouped q heads.

The head permutation `perm` is folded into the weights on the host:
  - Wq columns are reordered into group-major "slot" order (slot j = head
    perm[j]) and pre-scaled by HD**-0.5,
  - Wk/Wv rows are reordered by argsort(perm) (equivalent to shuffling x),
  - Wp rows are reordered by perm (undoes the output permutation).
so the device kernel is a plain GQA with no gathers.

On device (per core), matmuls run in fp32r (single-pass fp32, ~1e-4 rel):
  xT [768,2048] and the weight shards are DMA'd to SBUF once.  qT and a
  partition-duplicated kT are computed in transposed layout [hd, P]; v in
  natural layout [P, hd] with an appended ones column (the ones column makes
  the attention matmul also produce the softmax denominator).  Attention per
  (kv head, q head, q-block of 1024): scores_T = kT.T @ qT -> PSUM, exp on ACT
  (scores are O(7), no max subtraction needed) -> SBUF fp32r, then
  outT[65,1024] = [v|1].T @ exp_T accumulated over the 16 kv row tiles in
  PSUM (row 64 = softmax denominator).  Normalize via reciprocal row +
  DMA partition-broadcast + DVE multiply into per-head outT [64, P] tiles,
  then project with Wp as six K=64 accumulation steps.  Partial outputs are
  summed on the host with the bias.
"""

import numpy as np

B, P, C = 4, 2048, 768
H, HK, HD, GS = 12, 6, 64, 2
SCALE = HD ** -0.5
NKV = 3          # kv heads per core
NH = 6           # q heads per core
KT = C // 128    # 6 contraction tiles
PT = P // 128    # 16 row tiles
QB = 1024        # q-block width for attention
NQB = P // QB    # 2

_cached_nc = None


def _build_program():
    global _cached_nc
    if _cached_nc is not None:
        return _cached_nc

    import concourse.mybir as mybir
    import concourse.tile as tile
    from concourse import bacc

    fp32 = mybir.dt.float32
    fp32r = mybir.dt.float32r
    EXP = mybir.ActivationFunctionType.Exp

    nc = bacc.Bacc("TRN2", target_bir_lowering=False, debug=False)
    xT = nc.dram_tensor("xT", [C, P], fp32r, kind="ExternalInput").ap()
    wq = nc.dram_tensor("wq", [C, 384], fp32r, kind="ExternalInput").ap()
    wkd = nc.dram_tensor("wkd", [C, 384], fp32r, kind="ExternalInput").ap()
    wv = nc.dram_tensor("wv", [C, 192], fp32r, kind="ExternalInput").ap()
    wp = nc.dram_tensor("wp", [384, C], fp32r, kind="ExternalInput").ap()
    y = nc.dram_tensor("y", [P, C], fp32, kind="ExternalOutput").ap()
    # DRAM bounce buffer for partition-broadcasting the softmax reciprocal
    rcd = nc.dram_tensor("rcd", [NKV * GS * NQB, QB], fp32).ap()

    with tile.TileContext(nc) as tc:
        from contextlib import ExitStack

        with ExitStack() as ctx:
            wpool = ctx.enter_context(tc.tile_pool(name="weights", bufs=1))
            qkvp = ctx.enter_context(tc.tile_pool(name="qkv", bufs=1))
            ysbp = ctx.enter_context(tc.tile_pool(name="ysb", bufs=3))
            nrmp = ctx.enter_context(tc.tile_pool(name="norm", bufs=2))

            wq_sb = wpool.tile([128, KT, 384], fp32r)
            nc.sync.dma_start(wq_sb[:], wq.rearrange("(t p) n -> p t n", p=128))
            wkd_sb = wpool.tile([128, KT, 384], fp32r)
            nc.sync.dma_start(wkd_sb[:], wkd.rearrange("(t p) n -> p t n", p=128))
            wv_sb = wpool.tile([128, KT, 192], fp32r)
            nc.sync.dma_start(wv_sb[:], wv.rearrange("(t p) n -> p t n", p=128))
            wps = []
            for h in range(NH):
                w = wpool.tile([64, C], fp32r, name=f"wp{h}", tag=f"wp{h}")
                nc.sync.dma_start(w[:], wp[h * 64 : (h + 1) * 64, :])
                wps.append(w)

            qts = [qkvp.tile([128, P], fp32r, name=f"qt{m}", tag=f"qt{m}") for m in range(NKV)]
            kts = [qkvp.tile([128, P], fp32r, name=f"kt{m}", tag=f"kt{m}") for m in range(NKV)]
            vexts = [
                qkvp.tile([128, PT, HD + 1], fp32r, name=f"vx{m}", tag=f"vx{m}")
                for m in range(NKV)
            ]
            for m in range(NKV):
                nc.vector.memset(vexts[m][:, :, HD].bitcast(fp32), 1.0)

            # ---------------- phase 1: QKV projections ----------------
            with tc.tile_pool(name="xt", bufs=1) as xpool, tc.tile_pool(
                name="qkv_ps", bufs=3, space="PSUM"
            ) as qps:
                xt = xpool.tile([128, KT, P], fp32r)
                for kc in range(KT):
                    nc.sync.dma_start(xt[:, kc, :], xT[kc * 128 : (kc + 1) * 128, :])

                for w_sb, dests in ((wq_sb, qts), (wkd_sb, kts)):
                    for m in range(NKV):
                        for nb in range(4):
                            ps = qps.tile([128, 512], fp32, tag="proj")
                            for kc in range(KT):
                                nc.tensor.matmul(
                                    ps[:],
                                    w_sb[:, kc, m * 128 : (m + 1) * 128],
                                    xt[:, kc, nb * 512 : (nb + 1) * 512],
                                    start=(kc == 0),
                                    stop=(kc == KT - 1),
                                )
                            nc.vector.tensor_copy(
                                dests[m][:, nb * 512 : (nb + 1) * 512], ps[:]
                            )

                for i in range(PT):
                    ps = qps.tile([128, 192], fp32, tag="vproj")
                    for kc in range(KT):
                        nc.tensor.matmul(
                            ps[:],
                            xt[:, kc, i * 128 : (i + 1) * 128],
                            wv_sb[:, kc, :],
                            start=(kc == 0),
                            stop=(kc == KT - 1),
                        )
                    psv = ps.rearrange("p (h d) -> p h d", h=NKV)
                    for m in range(NKV):
                        nc.vector.tensor_copy(vexts[m][:, i, 0:HD], psv[:, m, :])

            # per-head transposed attention outputs [64, P] (after xt freed)
            with ExitStack() as octx:
                outp = octx.enter_context(tc.tile_pool(name="outT", bufs=1))
                outTs = [
                    outp.tile([64, P], fp32r, name=f"oT{h}", tag=f"oT{h}")
                    for h in range(NH)
                ]

                # ---------------- phase 2: attention ----------------
                with tc.tile_pool(name="E", bufs=3) as epool, tc.tile_pool(
                    name="s_ps", bufs=2, space="PSUM"
                ) as sps, tc.tile_pool(
                    name="o_ps", bufs=2, space="PSUM"
                ) as ops:
                    for kv in range(NKV):
                        for g in range(GS):
                            gp = slice(g * 64, (g + 1) * 64)
                            h = 2 * kv + g
                            for jq in range(NQB):
                                q0 = jq * QB
                                ob = ops.tile([HD + 1, QB], fp32, tag="ob")
                                for i in range(PT):
                                    s = sps.tile([128, QB], fp32, tag="s")
                                    for nb in range(QB // 512):
                                        nc.tensor.matmul(
                                            s[:, nb * 512 : (nb + 1) * 512],
                                            kts[kv][gp, i * 128 : (i + 1) * 128],
                                            qts[kv][gp, q0 + nb * 512 : q0 + (nb + 1) * 512],
                                            start=True,
                                            stop=True,
                                        )
                                    e = epool.tile([128, QB], fp32r, tag="e")
                                    nc.scalar.activation(e[:], s[:], EXP)
                                    for nb in range(QB // 512):
                                        nc.tensor.matmul(
                                            ob[:, nb * 512 : (nb + 1) * 512],
                                            vexts[kv][:, i, :],
                                            e[:, nb * 512 : (nb + 1) * 512],
                                            start=(i == 0),
                                            stop=(i == PT - 1),
                                        )
                                # normalize: row HD of ob is the softmax sum
                                rc = nrmp.tile([HD + 1, QB], fp32, tag="rc")
                                nc.vector.reciprocal(
                                    rc[HD : HD + 1, :], ob[HD : HD + 1, :]
                                )
                                u = (2 * kv + g) * NQB + jq
                                nc.sync.dma_start(rcd[u : u + 1, :], rc[HD : HD + 1, :])
                                rb = nrmp.tile([HD, QB], fp32, tag="rb")
                                import concourse.bass as bass

                                bcast = bass.AP(
                                    tensor=rcd.tensor,
                                    offset=u * QB,
                                    ap=[[0, HD], [1, QB]],
                                )
                                nc.gpsimd.dma_start(rb[:], bcast)
                                nc.vector.tensor_mul(
                                    outTs[h][:, q0 : q0 + QB], ob[0:HD, :], rb[:]
                                )

                # ---------------- phase 3: output projection ----------------
                with tc.tile_pool(name="y_ps", bufs=4, space="PSUM") as yps:
                    for mt in range(PT):
                        for nh in range(2):
                            ps = yps.tile([128, 384], fp32, tag="y")
                            for h in range(NH):
                                nc.tensor.matmul(
                                    ps[:],
                                    outTs[h][:, mt * 128 : (mt + 1) * 128],
                                    wps[h][:, nh * 384 : (nh + 1) * 384],
                                    start=(h == 0),
                                    stop=(h == NH - 1),
                                )
                            ysb = ysbp.tile([128, 384], fp32, tag="y")
                            nc.vector.tensor_copy(ysb[:], ps[:])
                            nc.sync.dma_start(
                                y[mt * 128 : (mt + 1) * 128, nh * 384 : (nh + 1) * 384],
                                ysb[:],
                            )

    nc.compile()
    _cached_nc = nc
    return nc


def _make_in_maps(x, Wq, Wk, Wv, Wp, perm):
    inv = np.argsort(perm)
    Wq_f = np.ascontiguousarray(
        Wq.reshape(C, H, HD)[:, perm, :].reshape(C, C) * SCALE
    )
    Wk_f = np.ascontiguousarray(Wk.reshape(H, HD, HK * HD)[inv].reshape(C, HK * HD))
    Wv_f = np.ascontiguousarray(Wv.reshape(H, HD, HK * HD)[inv].reshape(C, HK * HD))
    Wp_f = np.ascontiguousarray(Wp.reshape(H, HD, C)[perm].reshape(C, C))

    in_maps = []
    for core in range(8):
        b, half = core // 2, core % 2
        wk_half = Wk_f[:, half * 192 : (half + 1) * 192].reshape(C, NKV, 1, HD)
        wkd = np.ascontiguousarray(
            np.broadcast_to(wk_half, (C, NKV, 2, HD)).reshape(C, 384)
        )
        in_maps.append(
            {
                "xT": np.ascontiguousarray(x[b].T),
                "wq": np.ascontiguousarray(Wq_f[:, half * 384 : (half + 1) * 384]),
                "wkd": wkd,
                "wv": np.ascontiguousarray(Wv_f[:, half * 192 : (half + 1) * 192]),
                "wp": np.ascontiguousarray(Wp_f[half * 384 : (half + 1) * 384, :]),
            }
        )
    return in_maps


def kernel(x, Wq, Wk, Wv, Wp, bp, bass_run_kwargs=None, **_unused):
    perm = _unused.pop("perm")
    from concourse.bass_utils import run_bass_kernel_spmd

    x = np.asarray(x, np.float32)
    nc = _build_program()
    in_maps = _make_in_maps(
        x,
        np.asarray(Wq, np.float32),
        np.asarray(Wk, np.float32),
        np.asarray(Wv, np.float32),
        np.asarray(Wp, np.float32),
        np.asarray(perm),
    )
    res = run_bass_kernel_spmd(
        nc, in_maps, core_ids=list(range(8)), **(bass_run_kwargs or {})
    )
    bp = np.asarray(bp, np.float32)
    y = np.empty((B, P, C), np.float32)
    for b in range(B):
        y[b] = res.results[2 * b]["y"] + res.results[2 * b + 1]["y"] + bp
    if bass_run_kwargs:
        kernel.last_results = res
    return y


# revision 11
# speedup vs baseline: 2.1549x; 1.2400x over previous
"""Trainium2 Bass kernel for GQA (nn_GQA_28561532518475).

8 cores = 4 batches x 2 kv-head halves.  perm is folded into the weights on
the host (Wq cols -> slot order * scale, Wk/Wv rows by argsort(perm), Wp rows
by perm), so the device kernel is a plain GQA.  All matmuls run in fp32r
(single-pass fp32, ~1e-4 rel).

Per core: qT and a partition-duplicated kT in [hd, P] layout, v natural with
a ones column (so the attention matmul also emits the softmax denominator).
Attention per (kv, g, q-block): scores_T = kT.T @ qT -> PSUM -> exp on ACT ->
SBUF, outT[65,1024] = [v|1].T @ E accumulated in PSUM; normalize via
reciprocal row + DRAM-bounce partition broadcast + DVE multiply.  Each kv
head's q/k projection is emitted AFTER the previous head's attention so the
scheduler uses it as PE filler (keeps the HAM clock-gate warm).  Output
projection reads packed [128, P] outT pairs.  Host sums the two partial
outputs per batch and adds the bias.
"""

import numpy as np

B, P, C = 4, 2048, 768
H, HK, HD, GS = 12, 6, 64, 2
SCALE = HD ** -0.5
NKV = 3          # kv heads per core
NH = 6           # q heads per core
KT = C // 128    # 6 contraction tiles
PT = P // 128    # 16 row tiles
QB = 1024        # q-block width for attention
NQB = P // QB    # 2

_cached_nc = None


def _build_program():
    global _cached_nc
    if _cached_nc is not None:
        return _cached_nc

    import concourse.bass as bass
    import concourse.mybir as mybir
    import concourse.tile as tile
    from concourse import bacc

    fp32 = mybir.dt.float32
    fp32r = mybir.dt.float32r
    EXP = mybir.ActivationFunctionType.Exp

    nc = bacc.Bacc("TRN2", target_bir_lowering=False, debug=False)
    xT = nc.dram_tensor("xT", [C, P], fp32r, kind="ExternalInput").ap()
    wq = nc.dram_tensor("wq", [C, 384], fp32r, kind="ExternalInput").ap()
    wkd = nc.dram_tensor("wkd", [C, 384], fp32r, kind="ExternalInput").ap()
    wv = nc.dram_tensor("wv", [C, 192], fp32r, kind="ExternalInput").ap()
    wp = nc.dram_tensor("wp", [384, C], fp32r, kind="ExternalInput").ap()
    y = nc.dram_tensor("y", [P, C], fp32, kind="ExternalOutput").ap()
    rcd = nc.dram_tensor("rcd", [NH * NQB, QB], fp32).ap()

    with tile.TileContext(nc) as tc:
        from contextlib import ExitStack

        with ExitStack() as ctx:
            wpool = ctx.enter_context(tc.tile_pool(name="weights", bufs=1))
            qkvp = ctx.enter_context(tc.tile_pool(name="qkv", bufs=1))
            xpool = ctx.enter_context(tc.tile_pool(name="xt", bufs=1))
            outp = ctx.enter_context(tc.tile_pool(name="outT", bufs=1))
            epool = ctx.enter_context(tc.tile_pool(name="E", bufs=2))
            nrmp = ctx.enter_context(tc.tile_pool(name="norm", bufs=1))
            ysbp = ctx.enter_context(tc.tile_pool(name="ysb", bufs=2))

            wq_sb = wpool.tile([128, KT, 384], fp32r)
            nc.sync.dma_start(wq_sb[:], wq.rearrange("(t p) n -> p t n", p=128))
            wkd_sb = wpool.tile([128, KT, 384], fp32r)
            nc.sync.dma_start(wkd_sb[:], wkd.rearrange("(t p) n -> p t n", p=128))
            wv_sb = wpool.tile([128, KT, 192], fp32r)
            nc.sync.dma_start(wv_sb[:], wv.rearrange("(t p) n -> p t n", p=128))
            wp_sb = wpool.tile([128, 3, C], fp32r)
            nc.sync.dma_start(wp_sb[:], wp.rearrange("(t p) n -> p t n", p=128))

            qts = [qkvp.tile([128, P], fp32r, name=f"qt{m}", tag=f"qt{m}") for m in range(NKV)]
            kts = [qkvp.tile([128, P], fp32r, name=f"kt{m}", tag=f"kt{m}") for m in range(NKV)]
            vexts = [
                qkvp.tile([128, PT, HD + 1], fp32r, name=f"vx{m}", tag=f"vx{m}")
                for m in range(NKV)
            ]
            for m in range(NKV):
                nc.vector.memset(vexts[m][:, :, HD].bitcast(fp32), 1.0)
            outTs = [outp.tile([128, P], fp32r, name=f"oT{m}", tag=f"oT{m}") for m in range(NKV)]

            xt = xpool.tile([128, KT, P], fp32r)
            for kc in range(KT):
                for ch in range(2):
                    nc.sync.dma_start(
                        xt[:, kc, ch * 1024 : (ch + 1) * 1024],
                        xT[kc * 128 : (kc + 1) * 128, ch * 1024 : (ch + 1) * 1024],
                    )

            with tc.tile_pool(name="mm_ps", bufs=2, space="PSUM") as sps, tc.tile_pool(
                name="o_ps", bufs=2, space="PSUM"
            ) as ops:

                def qk_proj(kv):
                    for w_sb, dest in ((wq_sb, qts[kv]), (wkd_sb, kts[kv])):
                        for nb in range(4):
                            ps = sps.tile([128, 512], fp32, name="pj", tag="s")
                            for kc in range(KT):
                                nc.tensor.matmul(
                                    ps[:],
                                    w_sb[:, kc, kv * 128 : (kv + 1) * 128],
                                    xt[:, kc, nb * 512 : (nb + 1) * 512],
                                    start=(kc == 0),
                                    stop=(kc == KT - 1),
                                )
                            nc.vector.tensor_copy(
                                dest[:, nb * 512 : (nb + 1) * 512], ps[:]
                            )

                def v_proj_all():
                    for i in range(PT):
                        ps = sps.tile([128, 192], fp32, name="vp", tag="s")
                        for kc in range(KT):
                            nc.tensor.matmul(
                                ps[:],
                                xt[:, kc, i * 128 : (i + 1) * 128],
                                wv_sb[:, kc, :],
                                start=(kc == 0),
                                stop=(kc == KT - 1),
                            )
                        psv = ps.rearrange("p (h d) -> p h d", h=NKV)
                        for m in range(NKV):
                            nc.vector.tensor_copy(vexts[m][:, i, 0:HD], psv[:, m, :])

                def attention(kv):
                    for g in range(GS):
                        gp = slice(g * 64, (g + 1) * 64)
                        h = 2 * kv + g
                        for jq in range(NQB):
                            q0 = jq * QB
                            u = h * NQB + jq
                            ob = ops.tile([HD + 1, QB], fp32, tag="ob")
                            for i in range(PT):
                                s = sps.tile([128, QB], fp32, name="sc", tag="s")
                                for nb in range(QB // 512):
                                    nc.tensor.matmul(
                                        s[:, nb * 512 : (nb + 1) * 512],
                                        kts[kv][gp, i * 128 : (i + 1) * 128],
                                        qts[kv][gp, q0 + nb * 512 : q0 + (nb + 1) * 512],
                                        start=True,
                                        stop=True,
                                    )
                                e = epool.tile([128, QB], fp32r, tag="e")
                                nc.scalar.activation(e[:], s[:], EXP)
                                for nb in range(QB // 512):
                                    nc.tensor.matmul(
                                        ob[:, nb * 512 : (nb + 1) * 512],
                                        vexts[kv][:, i, :],
                                        e[:, nb * 512 : (nb + 1) * 512],
                                        start=(i == 0),
                                        stop=(i == PT - 1),
                                    )
                            rc = nrmp.tile([HD + 1, QB], fp32, tag="rc")
                            nc.vector.reciprocal(rc[HD : HD + 1, :], ob[HD : HD + 1, :])
                            nc.sync.dma_start(rcd[u : u + 1, :], rc[HD : HD + 1, :])
                            rb = nrmp.tile([HD, QB], fp32, tag="rb")
                            bcast = bass.AP(
                                tensor=rcd.tensor, offset=u * QB, ap=[[0, HD], [1, QB]]
                            )
                            nc.gpsimd.dma_start(rb[:], bcast)
                            if g == 0:
                                nc.vector.tensor_mul(
                                    outTs[kv][0:HD, q0 : q0 + QB], ob[0:HD, :], rb[:]
                                )
                            else:
                                sc2 = nrmp.tile([HD, QB], fp32r, tag="sc2")
                                nc.vector.tensor_mul(sc2[:], ob[0:HD, :], rb[:])
                                nc.sync.dma_start(
                                    outTs[kv][HD:128, q0 : q0 + QB], sc2[:]
                                )

                qk_proj(0)
                v_proj_all()
                attention(0)
                qk_proj(1)
                attention(1)
                qk_proj(2)
                attention(2)

            # ---------------- output projection ----------------
            with tc.tile_pool(name="y_ps", bufs=4, space="PSUM") as yps:
                for mt in range(PT):
                    for nh in range(2):
                        ps = yps.tile([128, 384], fp32, tag="y")
                        for kf in range(3):
                            nc.tensor.matmul(
                                ps[:],
                                outTs[kf][:, mt * 128 : (mt + 1) * 128],
                                wp_sb[:, kf, nh * 384 : (nh + 1) * 384],
                                start=(kf == 0),
                                stop=(kf == 2),
                            )
                        ysb = ysbp.tile([128, 384], fp32, tag="y")
                        nc.vector.tensor_copy(ysb[:], ps[:])
                        nc.sync.dma_start(
                            y[mt * 128 : (mt + 1) * 128, nh * 384 : (nh + 1) * 384],
                            ysb[:],
                        )

    nc.compile()
    _cached_nc = nc
    return nc


def _make_in_maps(x, Wq, Wk, Wv, Wp, perm):
    inv = np.argsort(perm)
    Wq_f = np.ascontiguousarray(
        Wq.reshape(C, H, HD)[:, perm, :].reshape(C, C) * SCALE
    )
    Wk_f = np.ascontiguousarray(Wk.reshape(H, HD, HK * HD)[inv].reshape(C, HK * HD))
    Wv_f = np.ascontiguousarray(Wv.reshape(H, HD, HK * HD)[inv].reshape(C, HK * HD))
    Wp_f = np.ascontiguousarray(Wp.reshape(H, HD, C)[perm].reshape(C, C))

    in_maps = []
    for core in range(8):
        b, half = core // 2, core % 2
        wk_half = Wk_f[:, half * 192 : (half + 1) * 192].reshape(C, NKV, 1, HD)
        wkd = np.ascontiguousarray(
            np.broadcast_to(wk_half, (C, NKV, 2, HD)).reshape(C, 384)
        )
        in_maps.append(
            {
                "xT": np.ascontiguousarray(x[b].T),
                "wq": np.ascontiguousarray(Wq_f[:, half * 384 : (half + 1) * 384]),
                "wkd": wkd,
                "wv": np.ascontiguousarray(Wv_f[:, half * 192 : (half + 1) * 192]),
                "wp": np.ascontiguousarray(Wp_f[half * 384 : (half + 1) * 384, :]),
            }
        )
    return in_maps


def kernel(x, Wq, Wk, Wv, Wp, bp, bass_run_kwargs=None, **_unused):
    perm = _unused.pop("perm")
    from concourse.bass_utils import run_bass_kernel_spmd

    x = np.asarray(x, np.float32)
    nc = _build_program()
    in_maps = _make_in_maps(
        x,
        np.asarray(Wq, np.float32),
        np.asarray(Wk, np.float32),
        np.asarray(Wv, np.float32),
        np.asarray(Wp, np.float32),
        np.asarray(perm),
    )
    res = run_bass_kernel_spmd(
        nc, in_maps, core_ids=list(range(8)), **(bass_run_kwargs or {})
    )
    bp = np.asarray(bp, np.float32)
    y = np.empty((B, P, C), np.float32)
    for b in range(B):
        y[b] = res.results[2 * b]["y"] + res.results[2 * b + 1]["y"] + bp
    if bass_run_kwargs:
        kernel.last_results = res
    return y


# revision 14
# speedup vs baseline: 2.5248x; 1.1717x over previous
"""Trainium2 Bass kernel for GQA (nn_GQA_28561532518475).

8 cores = 4 batches x 2 kv-head halves.  perm is folded into the weights on
the host (Wq cols -> slot order * scale, Wk/Wv rows by argsort(perm), Wp rows
by perm), so the device kernel is a plain GQA.  All matmuls run in fp32r
(single-pass fp32, ~1e-4 rel).

Per core: qT and a partition-duplicated kT in [hd, P] layout, v natural with
a ones column (so the attention matmul also emits the softmax denominator).
Attention per (kv, g, q-block): scores_T = kT.T @ qT -> PSUM -> exp on ACT ->
SBUF, outT[65,1024] = [v|1].T @ E accumulated in PSUM; normalize via
reciprocal row + DRAM-bounce partition broadcast + DVE multiply.  Each kv
head's q/k projection is emitted AFTER the previous head's attention so the
scheduler uses it as PE filler (keeps the HAM clock-gate warm).  Output
projection reads packed [128, P] outT pairs.  Host sums the two partial
outputs per batch and adds the bias.
"""

import numpy as np

B, P, C = 4, 2048, 768
H, HK, HD, GS = 12, 6, 64, 2
SCALE = HD ** -0.5
NKV = 3          # kv heads per core
NH = 6           # q heads per core
KT = C // 128    # 6 contraction tiles
PT = P // 128    # 16 row tiles
QB = 1024        # q-block width for attention
NQB = P // QB    # 2

_cached_nc = None


def _build_program():
    global _cached_nc
    if _cached_nc is not None:
        return _cached_nc

    import concourse.bass as bass
    import concourse.mybir as mybir
    import concourse.tile as tile
    from concourse import bacc

    fp32 = mybir.dt.float32
    fp32r = mybir.dt.float32r
    EXP = mybir.ActivationFunctionType.Exp

    nc = bacc.Bacc("TRN2", target_bir_lowering=False, debug=False)
    xT = nc.dram_tensor("xT", [C, P], fp32r, kind="ExternalInput").ap()
    wq = nc.dram_tensor("wq", [C, 384], fp32r, kind="ExternalInput").ap()
    wkd = nc.dram_tensor("wkd", [C, 384], fp32r, kind="ExternalInput").ap()
    wv = nc.dram_tensor("wv", [C, 192], fp32r, kind="ExternalInput").ap()
    wp = nc.dram_tensor("wp", [384, C], fp32r, kind="ExternalInput").ap()
    y = nc.dram_tensor("y", [P, C], fp32, kind="ExternalOutput").ap()
    rcd = nc.dram_tensor("rcd", [NH * NQB, QB], fp32).ap()
    rcd2 = nc.dram_tensor("rcd2", [NH * NQB, QB], fp32).ap()

    with tile.TileContext(nc) as tc:
        from contextlib import ExitStack

        with ExitStack() as ctx:
            wpool = ctx.enter_context(tc.tile_pool(name="weights", bufs=1))
            qkvp = ctx.enter_context(tc.tile_pool(name="qkv", bufs=1))
            xpool = ctx.enter_context(tc.tile_pool(name="xt", bufs=1))
            outp = ctx.enter_context(tc.tile_pool(name="outT", bufs=1))
            epool = ctx.enter_context(tc.tile_pool(name="E", bufs=2))
            nrmp = ctx.enter_context(tc.tile_pool(name="norm", bufs=1))
            ysbp = ctx.enter_context(tc.tile_pool(name="ysb", bufs=2))

            wq_sb = wpool.tile([128, KT, 384], fp32r)
            nc.sync.dma_start(wq_sb[:], wq.rearrange("(t p) n -> p t n", p=128))
            wkd_sb = wpool.tile([128, KT, 384], fp32r)
            nc.sync.dma_start(wkd_sb[:], wkd.rearrange("(t p) n -> p t n", p=128))
            wv_sb = wpool.tile([128, KT, 192], fp32r)
            nc.sync.dma_start(wv_sb[:], wv.rearrange("(t p) n -> p t n", p=128))
            wp_sb = wpool.tile([128, 3, C], fp32r)
            nc.sync.dma_start(wp_sb[:], wp.rearrange("(t p) n -> p t n", p=128))

            qts = [qkvp.tile([128, P], fp32r, name=f"qt{m}", tag=f"qt{m}") for m in range(NKV)]
            kts = [qkvp.tile([128, P], fp32r, name=f"kt{m}", tag=f"kt{m}") for m in range(NKV)]
            vexts = [
                qkvp.tile([128, PT, HD + 1], fp32r, name=f"vx{m}", tag=f"vx{m}")
                for m in range(NKV)
            ]
            for m in range(NKV):
                nc.vector.memset(vexts[m][:, :, HD].bitcast(fp32), 1.0)
            outTs = [outp.tile([128, P], fp32r, name=f"oT{m}", tag=f"oT{m}") for m in range(NKV)]

            xt = xpool.tile([128, KT, P], fp32r)
            for kc in range(KT):
                for ch in range(2):
                    nc.sync.dma_start(
                        xt[:, kc, ch * 1024 : (ch + 1) * 1024],
                        xT[kc * 128 : (kc + 1) * 128, ch * 1024 : (ch + 1) * 1024],
                    )

            with tc.tile_pool(name="mm_ps", bufs=2, space="PSUM") as sps, tc.tile_pool(
                name="o_ps", bufs=2, space="PSUM"
            ) as ops:

                def qk_proj(kv):
                    for w_sb, dest in ((wq_sb, qts[kv]), (wkd_sb, kts[kv])):
                        for nb in range(4):
                            ps = sps.tile([128, 512], fp32, name="pj", tag="s")
                            for kc in range(KT):
                                nc.tensor.matmul(
                                    ps[:],
                                    w_sb[:, kc, kv * 128 : (kv + 1) * 128],
                                    xt[:, kc, nb * 512 : (nb + 1) * 512],
                                    start=(kc == 0),
                                    stop=(kc == KT - 1),
                                )
                            nc.vector.tensor_copy(
                                dest[:, nb * 512 : (nb + 1) * 512], ps[:]
                            )

                def v_proj_all():
                    for i in range(PT):
                        ps = sps.tile([128, 192], fp32, name="vp", tag="s")
                        for kc in range(KT):
                            nc.tensor.matmul(
                                ps[:],
                                xt[:, kc, i * 128 : (i + 1) * 128],
                                wv_sb[:, kc, :],
                                start=(kc == 0),
                                stop=(kc == KT - 1),
                            )
                        psv = ps.rearrange("p (h d) -> p h d", h=NKV)
                        for m in range(NKV):
                            nc.vector.tensor_copy(vexts[m][:, i, 0:HD], psv[:, m, :])

                def attention(kv):
                    for g in range(GS):
                        gp = slice(g * 64, (g + 1) * 64)
                        h = 2 * kv + g
                        for jq in range(NQB):
                            q0 = jq * QB
                            u = h * NQB + jq
                            ob = ops.tile([HD + 1, QB], fp32, tag="ob")
                            for i in range(PT):
                                s = sps.tile([128, QB], fp32, name="sc", tag="s")
                                for nb in range(QB // 512):
                                    nc.tensor.matmul(
                                        s[:, nb * 512 : (nb + 1) * 512],
                                        kts[kv][gp, i * 128 : (i + 1) * 128],
                                        qts[kv][gp, q0 + nb * 512 : q0 + (nb + 1) * 512],
                                        start=True,
                                        stop=True,
                                    )
                                e = epool.tile([128, QB], fp32r, tag="e")
                                nc.scalar.activation(e[:], s[:], EXP)
                                for nb in range(QB // 512):
                                    nc.tensor.matmul(
                                        ob[:, nb * 512 : (nb + 1) * 512],
                                        vexts[kv][:, i, :],
                                        e[:, nb * 512 : (nb + 1) * 512],
                                        start=(i == 0),
                                        stop=(i == PT - 1),
                                    )
                            # softmax sums -> DRAM, read back spread over 128
                            # lanes, reciprocal there, bounce back, broadcast
                            rc = nrmp.tile([HD + 1, QB], fp32, tag="rc")
                            nc.vector.tensor_copy(rc[HD : HD + 1, :], ob[HD : HD + 1, :])
                            nc.sync.dma_start(rcd[u : u + 1, :], rc[HD : HD + 1, :])
                            rr = nrmp.tile([128, QB // 128], fp32, tag="rr")
                            lanes = bass.AP(
                                tensor=rcd.tensor,
                                offset=u * QB,
                                ap=[[QB // 128, 128], [1, QB // 128]],
                            )
                            nc.sync.dma_start(rr[:], lanes)
                            rr2 = nrmp.tile([128, QB // 128], fp32, tag="rr2")
                            nc.vector.reciprocal(rr2[:], rr[:])
                            lanes2 = bass.AP(
                                tensor=rcd2.tensor,
                                offset=u * QB,
                                ap=[[QB // 128, 128], [1, QB // 128]],
                            )
                            nc.sync.dma_start(lanes2, rr2[:])
                            rb = nrmp.tile([HD, QB], fp32, tag="rb")
                            bcast = bass.AP(
                                tensor=rcd2.tensor, offset=u * QB, ap=[[0, HD], [1, QB]]
                            )
                            nc.gpsimd.dma_start(rb[:], bcast)
                            if g == 0:
                                nc.vector.tensor_mul(
                                    outTs[kv][0:HD, q0 : q0 + QB], ob[0:HD, :], rb[:]
                                )
                            else:
                                sc2 = nrmp.tile([HD, QB], fp32r, tag="sc2")
                                nc.vector.tensor_mul(sc2[:], ob[0:HD, :], rb[:])
                                nc.sync.dma_start(
                                    outTs[kv][HD:128, q0 : q0 + QB], sc2[:]
                                )

                v_proj_all()
                qk_proj(0)
                attention(0)
                qk_proj(1)
                attention(1)
                qk_proj(2)
                attention(2)

            # ---------------- output projection ----------------
            with tc.tile_pool(name="y_ps", bufs=4, space="PSUM") as yps:
                for mt in range(PT):
                    for nh in range(2):
                        ps = yps.tile([128, 384], fp32, tag="y")
                        for kf in range(3):
                            nc.tensor.matmul(
                                ps[:],
                                outTs[kf][:, mt * 128 : (mt + 1) * 128],
                                wp_sb[:, kf, nh * 384 : (nh + 1) * 384],
                                start=(kf == 0),
                                stop=(kf == 2),
                            )
                        ysb = ysbp.tile([128, 384], fp32, tag="y")
                        nc.vector.tensor_copy(ysb[:], ps[:])
                        nc.sync.dma_start(
                            y[mt * 128 : (mt + 1) * 128, nh * 384 : (nh + 1) * 384],
                            ysb[:],
                        )

    nc.compile()
    _cached_nc = nc
    return nc


def _make_in_maps(x, Wq, Wk, Wv, Wp, perm):
    inv = np.argsort(perm)
    Wq_f = np.ascontiguousarray(
        Wq.reshape(C, H, HD)[:, perm, :].reshape(C, C) * SCALE
    )
    Wk_f = np.ascontiguousarray(Wk.reshape(H, HD, HK * HD)[inv].reshape(C, HK * HD))
    Wv_f = np.ascontiguousarray(Wv.reshape(H, HD, HK * HD)[inv].reshape(C, HK * HD))
    Wp_f = np.ascontiguousarray(Wp.reshape(H, HD, C)[perm].reshape(C, C))

    in_maps = []
    for core in range(8):
        b, half = core // 2, core % 2
        wk_half = Wk_f[:, half * 192 : (half + 1) * 192].reshape(C, NKV, 1, HD)
        wkd = np.ascontiguousarray(
            np.broadcast_to(wk_half, (C, NKV, 2, HD)).reshape(C, 384)
        )
        in_maps.append(
            {
                "xT": np.ascontiguousarray(x[b].T),
                "wq": np.ascontiguousarray(Wq_f[:, half * 384 : (half + 1) * 384]),
                "wkd": wkd,
                "wv": np.ascontiguousarray(Wv_f[:, half * 192 : (half + 1) * 192]),
                "wp": np.ascontiguousarray(Wp_f[half * 384 : (half + 1) * 384, :]),
            }
        )
    return in_maps


def kernel(x, Wq, Wk, Wv, Wp, bp, bass_run_kwargs=None, **_unused):
    perm = _unused.pop("perm")
    from concourse.bass_utils import run_bass_kernel_spmd

    x = np.asarray(x, np.float32)
    nc = _build_program()
    in_maps = _make_in_maps(
        x,
        np.asarray(Wq, np.float32),
        np.asarray(Wk, np.float32),
        np.asarray(Wv, np.float32),
        np.asarray(Wp, np.float32),
        np.asarray(perm),
    )
    res = run_bass_kernel_spmd(
        nc, in_maps, core_ids=list(range(8)), **(bass_run_kwargs or {})
    )
    bp = np.asarray(bp, np.float32)
    y = np.empty((B, P, C), np.float32)
    for b in range(B):
        y[b] = res.results[2 * b]["y"] + res.results[2 * b + 1]["y"] + bp
    if bass_run_kwargs:
        kernel.last_results = res
    return y
